# revision 22
# baseline (speedup 1.0000x reference)
"""Trainium2 Bass kernel for nn_BilinAndFwdComboVecComp.

Math (B=8, S=256, C=256, V=64):
  final[b,s,z,k] = tanh( sum_ij ctx[b,s,i] ctx[b,z,j] W'[i,j,k] + A[b,z,k] + Bt[b,s,k] )
where
  W'[i,j,k] = W[i,j,k] + (i==j) * linmul_w[k,i]          (folds the `mul` branch)
  A[b,z,k]  = ctx[b] @ (lin1_w+lindiff_w).T + (lin1_b + bias + linmul_b + lindiff_b)
  Bt[b,s,k] = ctx[b] @ (lin2_w-lindiff_w).T + lin2_b     (the `diff` branch is rank-1
                                                          per pair and merges into A/Bt)

Sharding: V split across the 8 cores (8 k-values per core). Each core:
  phase 1: tmp2[i,(k,z)] = sum_j Wt[j,(k,i)]-slices^T @ ctxT[j,z]   (W-stationary;
           PSUM drained by DVE/ACT copies in parallel)
  phase 2: out[s,(k,z)]  = ctxT[:,s]^T @ tmp2[:,(k,z)] + fold matmul
           (Bt via fp16-hi/lo delta rows, A via fp16-hi/lo ones rows), tanh on ACT
           (fp16 out), DMA to a (B,S,KV,S) scratch; host transposes/concats.
Matmuls run in fp16 (full PE rate, 1 col/cycle; the 320-matmul dense stream
measures ~216ns per N=512 matmul = the warm 2.4GHz roofline, LDWEIGHTS fully
hidden by the HW background weight path). Key scheduling facts baked in here:
  - fold contraction zero-padded 18 -> 128 rows: short-K (row_grp-tiled)
    matmuls stream ~110ns/slot slower and their LDWEIGHTS can't hide; only
    the 18 data rows are DMA'd, pad rows are zeroed once on GpSimd (pool),
    keeping DVE free for the phase-1 PSUM drains.
  - ctx is staged in DRAM pre-packed in the SBUF tile layout (1KB-contiguous
    DMA lines); the phase-1-critical loads (ctx pair 0, progressive wt
    column windows consumed kk-major) ride the fast HWDGE ring in
    consumption order, bulk loads are issued later on the SWDGE ring.
  - 13 warmup matmuls on a zero tile bridge engine-init + input-DMA latency
    so the HAM clock is at K=8/8 when real work starts, with no PE idle gap.
  - stores are batched 512KB per (b,sc) mid-kernel; the last batches split
    per-PSUM-tile (and the final one in half) to shorten the tail drain.
Measured 90.4-92.4us on an idle-cooled device (vs 97.7us session baseline);
a hot board P0-downclocks the PE ~2.4->2.0GHz and adds ~13us — run-to-run
deltas of that size are thermal, not code.
KERNEL_DTYPE=f32r env switches to float32r compute (lower error, slower).
"""

import numpy as np

B, S, C, V = 8, 256, 256, 64
NCORES = 8
KV = V // NCORES  # k-values per core
KF = 128          # fold contraction rows (18 used, zero-padded to full array)


def _host_prep(ctx, W, bias, lin1_w, lin1_b, lin2_w, lin2_b,
               linmul_w, linmul_b, lindiff_w, lindiff_b):
    f = np.float32
    ctx = np.asarray(ctx, f)
    Wp = np.array(W, f)
    Wp[np.arange(C), np.arange(C), :] += np.asarray(linmul_w, f).T
    Wt = Wp.transpose(1, 0, 2)  # [j, i, k]

    A = ctx @ (np.asarray(lin1_w, f) + np.asarray(lindiff_w, f)).T \
        + (np.asarray(lin1_b, f) + np.asarray(bias, f)
           + np.asarray(linmul_b, f) + np.asarray(lindiff_b, f))
    Bt = ctx @ (np.asarray(lin2_w, f) - np.asarray(lindiff_w, f)).T + np.asarray(lin2_b, f)

    # ctx packed in the phase-1 SBUF tile layout: [pair, jchunk, c, h, z]
    # (h = which batch of the pair) so each DMA line is 1KB contiguous
    ctxT = ctx.transpose(0, 2, 1)  # [B, C, S]
    ctxp = np.ascontiguousarray(
        ctxT.reshape(B // 2, 2, 2, 128, S)      # [pair, h, j, c, z]
            .transpose(0, 2, 3, 1, 4))          # [pair, j, c, h, z]

    # delta in (k, z) layout: row r is 1 over the z-block of plane k==r
    delta = np.zeros((KV, KV * S), f)
    for r in range(KV):
        delta[r, r * S:(r + 1) * S] = 1.0

    per_core = []
    for c in range(NCORES):
        ks = slice(c * KV, (c + 1) * KV)
        # wt layout: [j, kk*C + i]
        wt = np.ascontiguousarray(Wt[:, :, ks].transpose(0, 2, 1).reshape(C, KV * C))
        # fold contraction rows 0..17: Bt hi/lo via delta rows (exact in fp16
        # as hi + residual), A hi/lo via ones rows; rows 18..127 zero
        Btc = Bt[:, :, ks].transpose(2, 0, 1).reshape(KV, B * S)
        Btc_hi = Btc.astype(np.float16).astype(f)
        Ac = A[:, :, ks].transpose(0, 2, 1).reshape(B, KV * S)
        Ac_hi = Ac.astype(np.float16).astype(f)
        KD = 2 * KV + 2  # populated fold rows; rows KD..KF-1 are zero on-chip
        foldL = np.empty((KD, B * S), f)
        foldL[:KV] = Btc_hi
        foldL[KV:2 * KV] = Btc - Btc_hi
        foldL[2 * KV:] = 1.0
        foldR = np.empty((B, KD, KV * S), f)
        foldR[:, :KV, :] = delta[None]
        foldR[:, KV:2 * KV, :] = delta[None]
        foldR[:, 2 * KV, :] = Ac_hi
        foldR[:, 2 * KV + 1, :] = Ac - Ac_hi
        per_core.append({"ctxp": ctxp, "wt": wt, "foldL": foldL, "foldR": foldR})
    import os
    if os.environ.get("KERNEL_DTYPE", "f16") == "f16":
        per_core = [{k: v.astype(np.float16) for k, v in m.items()} for m in per_core]
    return per_core


def _build_program():
    import concourse.tile as tile
    import concourse.mybir as mybir
    from concourse import bacc
    from contextlib import ExitStack

    import os
    f32 = mybir.dt.float32
    f16 = mybir.dt.float16
    if os.environ.get("KERNEL_DTYPE", "f16") == "f32r":
        f16 = mybir.dt.float32r  # compute dtype for matmul operands
    TANH = mybir.ActivationFunctionType.Tanh

    KD = 2 * KV + 2
    nc = bacc.Bacc("TRN2", target_bir_lowering=False, debug=False)
    ctxp_d = nc.dram_tensor("ctxp", [B // 2, 2, 128, 2 * S], f16, kind="ExternalInput").ap()
    wt_d = nc.dram_tensor("wt", [C, KV * C], f16, kind="ExternalInput").ap()
    foldL_d = nc.dram_tensor("foldL", [KD, B * S], f16, kind="ExternalInput").ap()
    foldR_d = nc.dram_tensor("foldR", [B, KD, S * KV], f16, kind="ExternalInput").ap()
    # out scratch is (k, z)-ordered; the host transposes back to (z, k)
    out_d = nc.dram_tensor("out", [B, S, KV, S], mybir.dt.float16, kind="ExternalOutput").ap()

    with tile.TileContext(nc) as tc, ExitStack() as es:
        ctx_pool = es.enter_context(tc.tile_pool(name="ctxp", bufs=8))
        wt_pool = es.enter_context(tc.tile_pool(name="wtp", bufs=2))
        fl_pool = es.enter_context(tc.tile_pool(name="flp", bufs=1))
        fr_pool = es.enter_context(tc.tile_pool(name="frp", bufs=3))
        tmp2_pool = es.enter_context(tc.tile_pool(name="tmp2p", bufs=8))
        outs_pool = es.enter_context(tc.tile_pool(name="outsp", bufs=4))

        # warmup is emitted first so its wsrc memset leads the DVE queue —
        # the PE can start ramping the HAM clock at engine-init time
        def warmup(ps2_pool, tanh_pool):
            wsrc = es.enter_context(tc.tile_pool(name="warmp", bufs=1)).tile(
                [128, 512], f16, name="wsrc", bufs=1)
            nc.vector.memset(wsrc[:], 0.0)
            wps = ps2_pool.tile([128, 1024], f32, name="ps2")
            for i in range(13):
                nc.tensor.matmul(wps[:, (i % 2) * 512:(i % 2) * 512 + 512],
                                 wsrc[:, 0:128], wsrc[:], start=True, stop=True)
            # preload the tanh spline tables while the PE warms up, so the
            # ~1.5us ACT_TABLE_LOAD doesn't stall the first real tanh
            tt = tanh_pool.tile([128, 8], mybir.dt.float16, name="ttl", bufs=1)
            nc.scalar.activation(tt[:], wsrc[:, 0:8], TANH)

        # Input staging. Waits fire only at DMA completion, so the critical
        # path (ctx pair 0 + progressive wt column windows, consumed kk-major
        # by phase 1) rides the fast HWDGE (sync) ring in consumption order;
        # everything phase 2 / late-phase-1 needs goes to the software ring.
        ctxp_sb = {}

        def load_ctx_pair(p, eng):
            for j in range(2):
                t = ctx_pool.tile([128, 2 * S], f16, name=f"ctx_{p}_{j}", bufs=1)
                eng.dma_start(t[:], ctxp_d[p, j])
                ctxp_sb[p, j] = t

        def load_inputs():
            load_ctx_pair(0, nc.sync)
            wt_sb = [wt_pool.tile([128, KV * C], f16, name=f"wt_{j}", bufs=1)
                     for j in range(2)]
            windows = [(0, C), (C, 3 * C), (3 * C, 5 * C), (5 * C, 7 * C),
                       (7 * C, 8 * C)]
            for lo, hi in windows:
                for j in range(2):
                    nc.sync.dma_start(wt_sb[j][:, lo:hi],
                                      wt_d[j * 128:(j + 1) * 128, lo:hi])
            # fold operands are used with KF=128 contraction rows but only
            # KD=18 carry data: DMA just those; pad rows are zeroed once on
            # GpSimd (DVE must stay free for phase-1 drains; foldL pad must
            # be exact zeros — stationary operand; frt pad just non-NaN).
            # The pool-queue memsets also delay the pair-1/2/3 + fold DMA
            # issues, keeping early HBM bandwidth for the critical pair 0/wt
            # (pair 1 is first needed ~8us after the dense stream starts).
            foldL_sb = fl_pool.tile([KF, B * S], f16, name="foldL", bufs=1)
            nc.gpsimd.memset(foldL_sb[:], 0.0)
            nc.gpsimd.dma_start(foldL_sb[:KD, :], foldL_d[:])
            load_ctx_pair(1, nc.gpsimd)
            # 3 rotating foldR slots inside one persistent tile (partition
            # slices must be 32-aligned: zero it all, DMA rows 0:KD per slot)
            frt_sb = fr_pool.tile([128, 3 * S * KV], f16, name="frt", bufs=1)
            nc.gpsimd.memset(frt_sb[:], 0.0)
            load_ctx_pair(2, nc.gpsimd)
            load_ctx_pair(3, nc.gpsimd)
            return wt_sb, foldL_sb, frt_sb

        tmp2p = {}

        def phase1(pg, ps1_pool, copy_engines=("vector",), chs=(0, 1)):
            # kk-major so the wt columns are consumed left-to-right, matching
            # the progressive wt window DMAs
            ce = [0]
            for ch in chs:
                for p in pg:
                    tmp2p[p, ch] = tmp2_pool.tile([128, 2 * KV * S], f16, name="tmp2")
            for kk in range(KV):
                for ch in chs:  # i-chunk (output partition of tmp2)
                    ps = {}
                    for p in pg:
                        ps[p] = ps1_pool.tile([128, 2 * S], f32, name="ps1")
                    for j in range(2):  # contraction chunk
                        lhsT = wt_sb[j][:, kk * C + ch * 128: kk * C + ch * 128 + 128]
                        for p in pg:
                            nc.tensor.matmul(
                                ps[p][:], lhsT, ctxp_sb[p, j][:],
                                start=(j == 0), stop=(j == 1),
                            )
                    for p in pg:
                        # one copy per bank: psum (h, z) -> pair tile (h, kk, z)
                        dst = tmp2p[p, ch][:].rearrange("q (h k z) -> q h k z", h=2, k=KV)
                        src_ap = ps[p][:].rearrange("q (h z) -> q h z", h=2)
                        eng = copy_engines[ce[0] % len(copy_engines)]
                        ce[0] += 1
                        if eng == "vector":
                            nc.vector.tensor_copy(dst[:, :, kk, :], src_ap)
                        else:
                            nc.scalar.copy(dst[:, :, kk, :], src_ap)

        def phase2(bg, ps2_pool, split_store=False):
            for b in bg:
                frt = frt_sb[:, (b % 3) * S * KV:(b % 3 + 1) * S * KV]
                nc.sync.dma_start(frt[:KD, :], foldR_d[b])
                for sc in range(2):
                    hoff = (b % 2) * KV * S
                    lhsT3 = foldL_sb[:, b * S + sc * 128: b * S + sc * 128 + 128]
                    split = split_store
                    ot = outs_pool.tile([128, 2048], mybir.dt.float16, name="ot")
                    for t in range(2):  # double-bank psum tiles, 2 n-chunks each
                        pst = ps2_pool.tile([128, 1024], f32, name="ps2")
                        n0 = 2 * t
                        for st in range(2):  # contraction chunk; one LDW per 2 MMs
                            lhsT = ctxp_sb[b // 2, st][:, (b % 2) * S + sc * 128:
                                                       (b % 2) * S + sc * 128 + 128]
                            for n in (n0, n0 + 1):
                                nc.tensor.matmul(
                                    pst[:, (n % 2) * 512:(n % 2) * 512 + 512], lhsT,
                                    tmp2p[b // 2, st][:, hoff + n * 512:hoff + (n + 1) * 512],
                                    start=(st == 0), stop=False,
                                )
                        for n in (n0, n0 + 1):
                            nc.tensor.matmul(
                                pst[:, (n % 2) * 512:(n % 2) * 512 + 512], lhsT3,
                                frt[:, n * 512:(n + 1) * 512],
                                start=False, stop=True,
                            )
                        nc.scalar.activation(ot[:, t * 1024:(t + 1) * 1024], pst[:], TANH)
                        if split:
                            # tail: don't wait for both tanh halves before storing
                            if b == 7 and sc == 1 and t == 1:
                                # very last tile: halve it across both queues
                                # so the final queue drain is shortest
                                for hd in range(2):
                                    eng = nc.sync if hd == 0 else nc.gpsimd
                                    eng.dma_start(
                                        out_d[b, sc * 128:(sc + 1) * 128,
                                              4 + 2 * hd:6 + 2 * hd]
                                        .rearrange("s k z -> s (k z)"),
                                        ot[:, 1024 + hd * 512:1536 + hd * 512],
                                    )
                            else:
                                eng = nc.sync if (sc + t) % 2 == 0 else nc.gpsimd
                                eng.dma_start(
                                    out_d[b, sc * 128:(sc + 1) * 128, 4 * t:4 * t + 4]
                                    .rearrange("s k z -> s (k z)"),
                                    ot[:, t * 1024:(t + 1) * 1024],
                                )
                    if not split:
                        # one batched 512KB store per (b, sc): 4KB per partition
                        eng = nc.sync if (b + sc) % 2 == 0 else nc.gpsimd
                        eng.dma_start(
                            out_d[b, sc * 128:(sc + 1) * 128]
                            .rearrange("s k z -> s (k z)"),
                            ot[:],
                        )

        ps1_pool = es.enter_context(tc.tile_pool(name="ps1", bufs=4, space="PSUM"))
        ps2_pool = es.enter_context(tc.tile_pool(name="ps2", bufs=2, space="PSUM"))
        mix = ("vector", "scalar")
        warmup(ps2_pool, outs_pool)
        wt_sb, foldL_sb, frt_sb = load_inputs()
        phase1([0], ps1_pool, copy_engines=mix)
        phase2([0], ps2_pool)
        phase1([1], ps1_pool, copy_engines=mix)
        phase2([1], ps2_pool)
        phase2([2], ps2_pool)
        phase1([2], ps1_pool, copy_engines=mix)
        phase2([3], ps2_pool)
        phase1([3], ps1_pool, copy_engines=mix, chs=(0,))
        phase2([4], ps2_pool)
        phase1([3], ps1_pool, copy_engines=mix, chs=(1,))
        phase2([5], ps2_pool, split_store=True)
        phase2([6], ps2_pool, split_store=True)
        phase2([7], ps2_pool, split_store=True)

    nc.compile()
    return nc


def _install_profile_hook():
    """Register the NTFF profile hook that the image's boot skipped
    (antenv.axon_hooks shim is missing in this container)."""
    import sys as _sys
    import types as _types
    try:
        import antenv
        if "antenv.axon_hooks" not in _sys.modules:
            m = _types.ModuleType("antenv.axon_hooks")
            _h = [None]
            m.set_axon_ntff_profile_hook = lambda h: _h.__setitem__(0, h)
            m.get_axon_ntff_profile_hook = lambda: _h[0]
            _sys.modules["antenv.axon_hooks"] = m
            antenv.axon_hooks = m
        from antenv.axon_hooks import set_axon_ntff_profile_hook, get_axon_ntff_profile_hook
        if get_axon_ntff_profile_hook() is None:
            from trn_agent_boot.trn_boot import _ntff_profile_via_ctypes
            set_axon_ntff_profile_hook(_ntff_profile_via_ctypes("/opt/axon/libaxon_pjrt.so"))
    except Exception:
        pass


def run(inputs, trace=False, repeats=1):
    """Returns (full_output, BassKernelResults)."""
    from concourse.bass_utils import run_bass_kernel_spmd

    if trace:
        _install_profile_hook()
    per_core = _host_prep(**inputs)
    nc = _build_program()
    import os as _os
    _tc = [int(x) for x in _os.environ.get("KERNEL_TRACE_CORES", "0").split(",")]
    times = []
    for r in range(repeats):
        res = run_bass_kernel_spmd(nc, per_core, list(range(NCORES)), trace=trace,
                                   trace_cores=_tc if trace else None)
        if res.exec_time_ns is not None:
            times.append(res.exec_time_ns)
    if times:
        res.all_exec_times_ns = times
    # per-core scratch is (B, S, KV, S) with k-major planes: swap to (B,S,S,KV)
    out = np.concatenate(
        [res.results[c]["out"].astype(np.float32).transpose(0, 1, 3, 2)
         for c in range(NCORES)], axis=3)
    out = np.ascontiguousarray(out)
    return out, res


def kernel(**inputs) -> np.ndarray:
    out, _ = run(inputs, trace=False)
    return out


# revision 23
# speedup vs baseline: 1.0061x; 1.0061x over previous
"""Trainium2 Bass kernel for nn_BilinAndFwdComboVecComp.

Math (B=8, S=256, C=256, V=64):
  final[b,s,z,k] = tanh( sum_ij ctx[b,s,i] ctx[b,z,j] W'[i,j,k] + A[b,z,k] + Bt[b,s,k] )
where
  W'[i,j,k] = W[i,j,k] + (i==j) * linmul_w[k,i]          (folds the `mul` branch)
  A[b,z,k]  = ctx[b] @ (lin1_w+lindiff_w).T + (lin1_b + bias + linmul_b + lindiff_b)
  Bt[b,s,k] = ctx[b] @ (lin2_w-lindiff_w).T + lin2_b     (the `diff` branch is rank-1
                                                          per pair and merges into A/Bt)

Sharding: V split across the 8 cores (8 k-values per core). Each core:
  phase 1: tmp2[i,(k,z)] = sum_j Wt[j,(k,i)]-slices^T @ ctxT[j,z]   (W-stationary;
           PSUM drained by DVE/ACT copies in parallel)
  phase 2: out[s,(k,z)]  = ctxT[:,s]^T @ tmp2[:,(k,z)] + fold matmul
           (Bt via fp16-hi/lo delta rows, A via fp16-hi/lo ones rows), tanh on ACT
           (fp16 out), DMA to a (B,S,KV,S) scratch; host transposes/concats.
Matmuls run in fp16 (full PE rate, 1 col/cycle; the 320-matmul dense stream
measures ~216ns per N=512 matmul = the warm 2.4GHz roofline, LDWEIGHTS fully
hidden by the HW background weight path). Key scheduling facts baked in here:
  - fold contraction zero-padded 18 -> 128 rows: short-K (row_grp-tiled)
    matmuls stream ~110ns/slot slower and their LDWEIGHTS can't hide; only
    the 18 data rows are DMA'd, pad rows are zeroed once on GpSimd (pool),
    keeping DVE free for the phase-1 PSUM drains.
  - ctx is staged in DRAM pre-packed in the SBUF tile layout (1KB-contiguous
    DMA lines); the phase-1-critical loads (ctx pair 0, progressive wt
    column windows consumed kk-major) ride the fast HWDGE ring in
    consumption order, bulk loads are issued later on the SWDGE ring.
  - 13 warmup matmuls on a zero tile bridge engine-init + input-DMA latency
    so the HAM clock is at K=8/8 when real work starts, with no PE idle gap.
  - stores are batched 512KB per (b,sc) mid-kernel; the last batches split
    per-PSUM-tile (and the final one in half) to shorten the tail drain.
Measured 90.4-92.4us on an idle-cooled device (vs 97.7us session baseline);
a hot board P0-downclocks the PE ~2.4->2.0GHz and adds ~13us — run-to-run
deltas of that size are thermal, not code.
KERNEL_DTYPE=f32r env switches to float32r compute (lower error, slower).
"""

import numpy as np

B, S, C, V = 8, 256, 256, 64
NCORES = 8
KV = V // NCORES  # k-values per core
KF = 128          # fold contraction rows (18 used, zero-padded to full array)


def _host_prep(ctx, W, bias, lin1_w, lin1_b, lin2_w, lin2_b,
               linmul_w, linmul_b, lindiff_w, lindiff_b):
    f = np.float32
    ctx = np.asarray(ctx, f)
    Wp = np.array(W, f)
    Wp[np.arange(C), np.arange(C), :] += np.asarray(linmul_w, f).T
    Wt = Wp.transpose(1, 0, 2)  # [j, i, k]

    A = ctx @ (np.asarray(lin1_w, f) + np.asarray(lindiff_w, f)).T \
        + (np.asarray(lin1_b, f) + np.asarray(bias, f)
           + np.asarray(linmul_b, f) + np.asarray(lindiff_b, f))
    Bt = ctx @ (np.asarray(lin2_w, f) - np.asarray(lindiff_w, f)).T + np.asarray(lin2_b, f)

    # ctx packed in the phase-1 SBUF tile layout: [pair, jchunk, c, h, z]
    # (h = which batch of the pair) so each DMA line is 1KB contiguous
    ctxT = ctx.transpose(0, 2, 1)  # [B, C, S]
    ctxp = np.ascontiguousarray(
        ctxT.reshape(B // 2, 2, 2, 128, S)      # [pair, h, j, c, z]
            .transpose(0, 2, 3, 1, 4))          # [pair, j, c, h, z]

    # delta in (k, z) layout: row r is 1 over the z-block of plane k==r
    delta = np.zeros((KV, KV * S), f)
    for r in range(KV):
        delta[r, r * S:(r + 1) * S] = 1.0

    per_core = []
    for c in range(NCORES):
        ks = slice(c * KV, (c + 1) * KV)
        # wt layout: [j, kk*C + i]
        wt = np.ascontiguousarray(Wt[:, :, ks].transpose(0, 2, 1).reshape(C, KV * C))
        # fold contraction rows 0..17: Bt hi/lo via delta rows (exact in fp16
        # as hi + residual), A hi/lo via ones rows; rows 18..127 zero
        Btc = Bt[:, :, ks].transpose(2, 0, 1).reshape(KV, B * S)
        Btc_hi = Btc.astype(np.float16).astype(f)
        Ac = A[:, :, ks].transpose(0, 2, 1).reshape(B, KV * S)
        Ac_hi = Ac.astype(np.float16).astype(f)
        KD = 2 * KV + 2  # populated fold rows; rows KD..KF-1 are zero on-chip
        foldL = np.empty((KD, B * S), f)
        foldL[:KV] = Btc_hi
        foldL[KV:2 * KV] = Btc - Btc_hi
        foldL[2 * KV:] = 1.0
        foldR = np.empty((B, KD, KV * S), f)
        foldR[:, :KV, :] = delta[None]
        foldR[:, KV:2 * KV, :] = delta[None]
        foldR[:, 2 * KV, :] = Ac_hi
        foldR[:, 2 * KV + 1, :] = Ac - Ac_hi
        per_core.append({"ctxp": ctxp, "wt": wt, "foldL": foldL, "foldR": foldR})
    import os
    if os.environ.get("KERNEL_DTYPE", "f16") == "f16":
        per_core = [{k: v.astype(np.float16) for k, v in m.items()} for m in per_core]
    return per_core


def _build_program():
    import concourse.tile as tile
    import concourse.mybir as mybir
    from concourse import bacc
    from contextlib import ExitStack

    import os
    f32 = mybir.dt.float32
    f16 = mybir.dt.float16
    if os.environ.get("KERNEL_DTYPE", "f16") == "f32r":
        f16 = mybir.dt.float32r  # compute dtype for matmul operands
    TANH = mybir.ActivationFunctionType.Tanh

    KD = 2 * KV + 2
    nc = bacc.Bacc("TRN2", target_bir_lowering=False, debug=False)
    ctxp_d = nc.dram_tensor("ctxp", [B // 2, 2, 128, 2 * S], f16, kind="ExternalInput").ap()
    wt_d = nc.dram_tensor("wt", [C, KV * C], f16, kind="ExternalInput").ap()
    foldL_d = nc.dram_tensor("foldL", [KD, B * S], f16, kind="ExternalInput").ap()
    foldR_d = nc.dram_tensor("foldR", [B, KD, S * KV], f16, kind="ExternalInput").ap()
    # out scratch is (k, z)-ordered; the host transposes back to (z, k)
    out_d = nc.dram_tensor("out", [B, S, KV, S], mybir.dt.float16, kind="ExternalOutput").ap()

    with tile.TileContext(nc) as tc, ExitStack() as es:
        ctx_pool = es.enter_context(tc.tile_pool(name="ctxp", bufs=8))
        wt_pool = es.enter_context(tc.tile_pool(name="wtp", bufs=2))
        fl_pool = es.enter_context(tc.tile_pool(name="flp", bufs=1))
        fr_pool = es.enter_context(tc.tile_pool(name="frp", bufs=3))
        tmp2_pool = es.enter_context(tc.tile_pool(name="tmp2p", bufs=8))
        outs_pool = es.enter_context(tc.tile_pool(name="outsp", bufs=4))

        # warmup is emitted first so its wsrc memset leads the DVE queue —
        # the PE can start ramping the HAM clock at engine-init time
        def warmup(ps2_pool, tanh_pool):
            wsrc = es.enter_context(tc.tile_pool(name="warmp", bufs=1)).tile(
                [128, 512], f16, name="wsrc", bufs=1)
            nc.vector.memset(wsrc[:], 0.0)
            wps = ps2_pool.tile([128, 1024], f32, name="ps2")
            for i in range(13):
                nc.tensor.matmul(wps[:, (i % 2) * 512:(i % 2) * 512 + 512],
                                 wsrc[:, 0:128], wsrc[:], start=True, stop=True)
            # preload the tanh spline tables while the PE warms up, so the
            # ~1.5us ACT_TABLE_LOAD doesn't stall the first real tanh
            tt = tanh_pool.tile([128, 8], mybir.dt.float16, name="ttl", bufs=1)
            nc.scalar.activation(tt[:], wsrc[:, 0:8], TANH)

        # Input staging. Waits fire only at DMA completion, so the critical
        # path (ctx pair 0 + progressive wt column windows, consumed kk-major
        # by phase 1) rides the fast HWDGE (sync) ring in consumption order;
        # everything phase 2 / late-phase-1 needs goes to the software ring.
        ctxp_sb = {}

        def load_ctx_pair(p, eng):
            for j in range(2):
                t = ctx_pool.tile([128, 2 * S], f16, name=f"ctx_{p}_{j}", bufs=1)
                eng.dma_start(t[:], ctxp_d[p, j])
                ctxp_sb[p, j] = t

        def load_inputs():
            load_ctx_pair(0, nc.sync)
            wt_sb = [wt_pool.tile([128, KV * C], f16, name=f"wt_{j}", bufs=1)
                     for j in range(2)]
            windows = [(0, C), (C, 3 * C), (3 * C, 5 * C), (5 * C, 7 * C),
                       (7 * C, 8 * C)]
            for lo, hi in windows:
                for j in range(2):
                    nc.sync.dma_start(wt_sb[j][:, lo:hi],
                                      wt_d[j * 128:(j + 1) * 128, lo:hi])
            # fold operands are used with KF=128 contraction rows but only
            # KD=18 carry data: DMA just those; pad rows are zeroed once on
            # GpSimd (DVE must stay free for phase-1 drains; foldL pad must
            # be exact zeros — stationary operand; frt pad just non-NaN).
            # The pool-queue memsets also delay the pair-1/2/3 + fold DMA
            # issues, keeping early HBM bandwidth for the critical pair 0/wt
            # (pair 1 is first needed ~8us after the dense stream starts).
            foldL_sb = fl_pool.tile([KF, B * S], f16, name="foldL", bufs=1)
            nc.gpsimd.memset(foldL_sb[:], 0.0)
            nc.gpsimd.dma_start(foldL_sb[:KD, :], foldL_d[:])
            load_ctx_pair(1, nc.gpsimd)
            # 3 rotating foldR slots inside one persistent tile (partition
            # slices must be 32-aligned: zero it all, DMA rows 0:KD per slot)
            frt_sb = fr_pool.tile([128, 3 * S * KV], f16, name="frt", bufs=1)
            nc.gpsimd.memset(frt_sb[:], 0.0)
            load_ctx_pair(2, nc.gpsimd)
            load_ctx_pair(3, nc.gpsimd)
            return wt_sb, foldL_sb, frt_sb

        tmp2p = {}

        def phase1(pg, ps1_pool, copy_engines=("vector",), chs=(0, 1)):
            # kk-major so the wt columns are consumed left-to-right, matching
            # the progressive wt window DMAs
            ce = [0]
            for ch in chs:
                for p in pg:
                    tmp2p[p, ch] = tmp2_pool.tile([128, 2 * KV * S], f16, name="tmp2")
            for kk in range(KV):
                for ch in chs:  # i-chunk (output partition of tmp2)
                    ps = {}
                    for p in pg:
                        ps[p] = ps1_pool.tile([128, 2 * S], f32, name="ps1")
                    for j in range(2):  # contraction chunk
                        lhsT = wt_sb[j][:, kk * C + ch * 128: kk * C + ch * 128 + 128]
                        for p in pg:
                            nc.tensor.matmul(
                                ps[p][:], lhsT, ctxp_sb[p, j][:],
                                start=(j == 0), stop=(j == 1),
                            )
                    for p in pg:
                        # one copy per bank: psum (h, z) -> pair tile (h, kk, z)
                        dst = tmp2p[p, ch][:].rearrange("q (h k z) -> q h k z", h=2, k=KV)
                        src_ap = ps[p][:].rearrange("q (h z) -> q h z", h=2)
                        eng = copy_engines[ce[0] % len(copy_engines)]
                        ce[0] += 1
                        if eng == "vector":
                            nc.vector.tensor_copy(dst[:, :, kk, :], src_ap)
                        else:
                            nc.scalar.copy(dst[:, :, kk, :], src_ap)

        def phase2(bg, ps2_pool, split_store=False):
            for b in bg:
                frt = frt_sb[:, (b % 3) * S * KV:(b % 3 + 1) * S * KV]
                nc.sync.dma_start(frt[:KD, :], foldR_d[b])
                for sc in range(2):
                    hoff = (b % 2) * KV * S
                    lhsT3 = foldL_sb[:, b * S + sc * 128: b * S + sc * 128 + 128]
                    split = split_store
                    ot = outs_pool.tile([128, 2048], mybir.dt.float16, name="ot")
                    for t in range(2):  # double-bank psum tiles, 2 n-chunks each
                        pst = ps2_pool.tile([128, 1024], f32, name="ps2")
                        n0 = 2 * t
                        for st in range(2):  # contraction chunk; one LDW per 2 MMs
                            lhsT = ctxp_sb[b // 2, st][:, (b % 2) * S + sc * 128:
                                                       (b % 2) * S + sc * 128 + 128]
                            for n in (n0, n0 + 1):
                                nc.tensor.matmul(
                                    pst[:, (n % 2) * 512:(n % 2) * 512 + 512], lhsT,
                                    tmp2p[b // 2, st][:, hoff + n * 512:hoff + (n + 1) * 512],
                                    start=(st == 0), stop=False,
                                )
                        for n in (n0, n0 + 1):
                            nc.tensor.matmul(
                                pst[:, (n % 2) * 512:(n % 2) * 512 + 512], lhsT3,
                                frt[:, n * 512:(n + 1) * 512],
                                start=False, stop=True,
                            )
                        if split and b == 7 and sc == 1 and t == 1:
                            # very last tile: split the tanh per psum bank so
                            # each half-store starts as soon as its half is done
                            for hd in range(2):
                                nc.scalar.activation(
                                    ot[:, 1024 + hd * 512:1536 + hd * 512],
                                    pst[:, hd * 512:(hd + 1) * 512], TANH)
                        else:
                            nc.scalar.activation(ot[:, t * 1024:(t + 1) * 1024], pst[:], TANH)
                        if split:
                            # tail: don't wait for both tanh halves before storing
                            if b == 7 and sc == 1 and t == 1:
                                # very last tile: halve it across both queues
                                # so the final queue drain is shortest
                                for hd in range(2):
                                    eng = nc.sync if hd == 0 else nc.gpsimd
                                    eng.dma_start(
                                        out_d[b, sc * 128:(sc + 1) * 128,
                                              4 + 2 * hd:6 + 2 * hd]
                                        .rearrange("s k z -> s (k z)"),
                                        ot[:, 1024 + hd * 512:1536 + hd * 512],
                                    )
                            else:
                                eng = nc.sync if (sc + t) % 2 == 0 else nc.gpsimd
                                eng.dma_start(
                                    out_d[b, sc * 128:(sc + 1) * 128, 4 * t:4 * t + 4]
                                    .rearrange("s k z -> s (k z)"),
                                    ot[:, t * 1024:(t + 1) * 1024],
                                )
                    if not split:
                        # one batched 512KB store per (b, sc): 4KB per partition
                        eng = nc.sync if (b + sc) % 2 == 0 else nc.gpsimd
                        eng.dma_start(
                            out_d[b, sc * 128:(sc + 1) * 128]
                            .rearrange("s k z -> s (k z)"),
                            ot[:],
                        )

        ps1_pool = es.enter_context(tc.tile_pool(name="ps1", bufs=4, space="PSUM"))
        ps2_pool = es.enter_context(tc.tile_pool(name="ps2", bufs=2, space="PSUM"))
        mix = ("vector", "scalar")
        warmup(ps2_pool, outs_pool)
        wt_sb, foldL_sb, frt_sb = load_inputs()
        phase1([0], ps1_pool, copy_engines=mix)
        phase2([0], ps2_pool)
        phase1([1], ps1_pool, copy_engines=mix)
        phase2([1], ps2_pool)
        phase2([2], ps2_pool)
        phase1([2], ps1_pool, copy_engines=mix)
        phase2([3], ps2_pool)
        phase1([3], ps1_pool, copy_engines=mix, chs=(0,))
        phase2([4], ps2_pool)
        phase1([3], ps1_pool, copy_engines=mix, chs=(1,))
        phase2([5], ps2_pool, split_store=True)
        phase2([6], ps2_pool, split_store=True)
        phase2([7], ps2_pool, split_store=True)

    nc.compile()
    return nc


def _install_profile_hook():
    """Register the NTFF profile hook that the image's boot skipped
    (antenv.axon_hooks shim is missing in this container)."""
    import sys as _sys
    import types as _types
    try:
        import antenv
        if "antenv.axon_hooks" not in _sys.modules:
            m = _types.ModuleType("antenv.axon_hooks")
            _h = [None]
            m.set_axon_ntff_profile_hook = lambda h: _h.__setitem__(0, h)
            m.get_axon_ntff_profile_hook = lambda: _h[0]
            _sys.modules["antenv.axon_hooks"] = m
            antenv.axon_hooks = m
        from antenv.axon_hooks import set_axon_ntff_profile_hook, get_axon_ntff_profile_hook
        if get_axon_ntff_profile_hook() is None:
            from trn_agent_boot.trn_boot import _ntff_profile_via_ctypes
            set_axon_ntff_profile_hook(_ntff_profile_via_ctypes("/opt/axon/libaxon_pjrt.so"))
    except Exception:
        pass


def run(inputs, trace=False, repeats=1):
    """Returns (full_output, BassKernelResults)."""
    from concourse.bass_utils import run_bass_kernel_spmd

    if trace:
        _install_profile_hook()
    per_core = _host_prep(**inputs)
    nc = _build_program()
    import os as _os
    _tc = [int(x) for x in _os.environ.get("KERNEL_TRACE_CORES", "0").split(",")]
    times = []
    for r in range(repeats):
        res = run_bass_kernel_spmd(nc, per_core, list(range(NCORES)), trace=trace,
                                   trace_cores=_tc if trace else None)
        if res.exec_time_ns is not None:
            times.append(res.exec_time_ns)
    if times:
        res.all_exec_times_ns = times
    # per-core scratch is (B, S, KV, S) with k-major planes: swap to (B,S,S,KV)
    out = np.concatenate(
        [res.results[c]["out"].astype(np.float32).transpose(0, 1, 3, 2)
         for c in range(NCORES)], axis=3)
    out = np.ascontiguousarray(out)
    return out, res


def kernel(**inputs) -> np.ndarray:
    out, _ = run(inputs, trace=False)
    return out
